# revision 2
# baseline (speedup 1.0000x reference)
"""MoE (top-2 of 8 experts) SwiGLU FFN on 8 Trainium2 NeuronCores.

Strategy (expert-parallel, per the sharding hint):
  - Router (x @ w_gate -> softmax -> top-2) computed host-side on jax-CPU with
    the exact ops the reference uses, so expert selection matches the
    reference bit-for-bit. This is the "dispatch tokens by topk_idx" step.
  - Core e receives only the tokens routed to expert e (gathered, transposed,
    and pre-cast to bf16 host-side), plus expert e's weights pre-packed into
    the SBUF tile layout (so every device DMA is a single contiguous 2D
    HWDGE transfer). All cores run one SPMD program sized to
    cap = max tokens per expert (zero-padded).
  - Device computes y_e^T = wo_e^T @ (silu(wg_e^T x^T) * (wi_e^T x^T)) with
    bf16 matmuls accumulating in fp32 PSUM. Tokens stay on the PSUM free
    dimension throughout, so no on-device transposes are needed: lhsT
    operands are the natural wi/wg [C,H] and wo [H,C] layouts.
  - Host combines: out[t] = val0[t]*y_{e0}[t] + val1[t]*y_{e1}[t].
"""

import numpy as np
import ml_dtypes

import concourse.bass as bass
import concourse.mybir as mybir
import concourse.tile as tile
from concourse.bass_utils import run_bass_kernel_spmd

N_CORES = 8
N_EXPERTS = 8
TOP_K = 2
B, T, C, H = 4, 2048, 1024, 2048
CC = C // 128           # contraction chunks over C
HH = H // 128           # chunks over H
TOK_TILE = 512          # tokens per PSUM tile (one fp32 bank)
HBW = 512               # stage-1 weight block width (columns of H)
CBW = 512               # stage-2 weight block width (columns of C)
HB = H // HBW
CB = C // CBW
BF16 = mybir.dt.bfloat16


def _split_multi_waits(nc, max_waits=1):
    """This walrus build rejects >1 sync-wait per instruction. Peel extra
    waits onto single-wait EventSemaphore instructions inserted just before,
    on the same engine (identical blocking semantics)."""
    n_split = 0
    for fn in nc.m.functions:
        for bb in fn.blocks:
            out = []
            changed = False
            for inst in bb.instructions:
                si = inst.sync_info
                waits = list(si.on_wait) if si is not None else []
                if len(waits) > max_waits:
                    head, keep = waits[:-max_waits], waits[-max_waits:]
                    for j, w in enumerate(head):
                        out.append(mybir.InstEventSemaphore(
                            name=f"{inst.name}-wspl{j}",
                            engine=inst.engine,
                            sync_info=mybir.SyncInfo(on_wait=[w], on_update=[]),
                        ))
                    inst.sync_info = mybir.SyncInfo(
                        on_wait=keep, on_update=list(si.on_update))
                    changed = True
                    n_split += 1
                out.append(inst)
            if changed:
                bb.instructions = out
    return n_split


def build_program(cap, reps=1):
    """One SPMD program: expert FFN over [cap] tokens (token dim = PSUM free
    dim everywhere). reps>1 repeats the whole compute (timing only).

    DRAM inputs are already in SBUF tile layout, bf16:
      xtb [128, CC*cap]     xtb[p, cc*cap+t]    = x^T[cc*128+p, t]
      wib [128, HB*CC*HBW]  wib[p, (hb*CC+cc)*HBW+f] = wi[cc*128+p, hb*HBW+f]
      wgb [128, HB*CC*HBW]  same layout as wib
      wob [128, CB*HH*CBW]  wob[p, (cb*HH+hh)*CBW+f] = wo[hh*128+p, cb*CBW+f]
    Output yt [C, cap] fp32 (y^T, one row block per c-chunk).
    """
    assert cap % 4 == 0
    nc = bass.Bass()
    xtb = nc.dram_tensor("xtb", [128, CC * cap], BF16, kind="ExternalInput")
    wib = nc.dram_tensor("wib", [128, HB * CC * HBW], BF16, kind="ExternalInput")
    wgb = nc.dram_tensor("wgb", [128, HB * CC * HBW], BF16, kind="ExternalInput")
    wob = nc.dram_tensor("wob", [128, CB * HH * CBW], BF16, kind="ExternalInput")
    yt = nc.dram_tensor("yt", [C, cap], mybir.dt.float32, kind="ExternalOutput")

    tok_tiles = [(t0, min(TOK_TILE, cap - t0)) for t0 in range(0, cap, TOK_TILE)]
    BLK = CC * HBW          # elements per (hb, cc-full) stage-1 block
    BLK2 = HH * CBW         # elements per (cb, hh-full) stage-2 block

    with tile.TileContext(nc) as tc:
        with tc.tile_pool(name="xb", bufs=1) as xb_pool, \
             tc.tile_pool(name="w1", bufs=2) as w1_pool, \
             tc.tile_pool(name="hT", bufs=1) as h_pool, \
             tc.tile_pool(name="w2", bufs=2) as w2_pool, \
             tc.tile_pool(name="sg", bufs=3) as sg_pool, \
             tc.tile_pool(name="yo", bufs=3) as yo_pool, \
             tc.tile_pool(name="ps", bufs=2, space="PSUM") as ps_pool, \
             tc.tile_pool(name="ps2", bufs=3, space="PSUM") as ps2_pool:

            for _rep in range(reps):
                xb = xb_pool.tile([128, CC * cap], BF16, tag="xb")
                nc.sync.dma_start(xb[:], xtb[:])

                # hT = silu(x@wg) * (x@wi), transposed: [H, cap] bf16
                hT = h_pool.tile([128, HH * cap], BF16, tag="hT")

                # ---- stage 1 ----
                for hb in range(HB):
                    wib_t = w1_pool.tile([128, BLK], BF16, tag="wib")
                    nc.sync.dma_start(wib_t[:],
                                      wib[:, hb * BLK:(hb + 1) * BLK])
                    wgb_t = w1_pool.tile([128, BLK], BF16, tag="wgb")
                    nc.sync.dma_start(wgb_t[:],
                                      wgb[:, hb * BLK:(hb + 1) * BLK])
                    for hi in range(HBW // 128):
                        hh = hb * (HBW // 128) + hi
                        for t0, tw in tok_tiles:
                            ps_u = ps_pool.tile([128, TOK_TILE],
                                                mybir.dt.float32, tag="psu")
                            ps_g = ps_pool.tile([128, TOK_TILE],
                                                mybir.dt.float32, tag="psg")
                            for cc in range(CC):
                                nc.tensor.matmul(
                                    ps_u[:, :tw],
                                    wib_t[:, cc * HBW + hi * 128:
                                          cc * HBW + (hi + 1) * 128],
                                    xb[:, cc * cap + t0: cc * cap + t0 + tw],
                                    start=(cc == 0), stop=(cc == CC - 1))
                            for cc in range(CC):
                                nc.tensor.matmul(
                                    ps_g[:, :tw],
                                    wgb_t[:, cc * HBW + hi * 128:
                                          cc * HBW + (hi + 1) * 128],
                                    xb[:, cc * cap + t0: cc * cap + t0 + tw],
                                    start=(cc == 0), stop=(cc == CC - 1))
                            sg = sg_pool.tile([128, TOK_TILE],
                                              mybir.dt.float32, tag="sg")
                            nc.scalar.activation(
                                sg[:, :tw], ps_g[:, :tw],
                                mybir.ActivationFunctionType.Silu)
                            nc.vector.tensor_mul(
                                hT[:, hh * cap + t0: hh * cap + t0 + tw],
                                ps_u[:, :tw], sg[:, :tw])

                # ---- stage 2: yT = wo^T @ hT ----
                for cb in range(CB):
                    wob_t = w2_pool.tile([128, BLK2], BF16, tag="wob")
                    nc.sync.dma_start(wob_t[:],
                                      wob[:, cb * BLK2:(cb + 1) * BLK2])
                    for ci in range(CBW // 128):
                        c0 = cb * CBW + ci * 128
                        for t0, tw in tok_tiles:
                            ps_y = ps2_pool.tile([128, TOK_TILE],
                                                 mybir.dt.float32, tag="psy")
                            for hh in range(HH):
                                nc.tensor.matmul(
                                    ps_y[:, :tw],
                                    wob_t[:, hh * CBW + ci * 128:
                                          hh * CBW + (ci + 1) * 128],
                                    hT[:, hh * cap + t0: hh * cap + t0 + tw],
                                    start=(hh == 0), stop=(hh == HH - 1))
                            yo = yo_pool.tile([128, TOK_TILE],
                                              mybir.dt.float32, tag="yo")
                            nc.vector.tensor_copy(yo[:, :tw], ps_y[:, :tw])
                            nc.sync.dma_start(yt[c0:c0 + 128, t0:t0 + tw],
                                              yo[:, :tw])
    _split_multi_waits(nc)
    return nc


def pack_wi(w):
    """wi/wg [C, H] f32 -> [128, HB*CC*HBW] bf16 in the wib DRAM layout."""
    a = np.asarray(w).reshape(CC, 128, HB, HBW)          # [cc, p, hb, f]
    a = a.transpose(1, 2, 0, 3)                          # [p, hb, cc, f]
    return np.ascontiguousarray(a.reshape(128, HB * CC * HBW)
                                ).astype(ml_dtypes.bfloat16)


def pack_wo(w):
    """wo [H, C] f32 -> [128, CB*HH*CBW] bf16 in the wob DRAM layout."""
    a = np.asarray(w).reshape(HH, 128, CB, CBW)          # [hh, p, cb, f]
    a = a.transpose(1, 2, 0, 3)                          # [p, cb, hh, f]
    return np.ascontiguousarray(a.reshape(128, CB * HH * CBW)
                                ).astype(ml_dtypes.bfloat16)


def pack_x(x_disp_T):
    """x^T dispatch slab [C, cap] f32 -> [128, CC*cap] bf16 (xtb layout)."""
    cap = x_disp_T.shape[1]
    a = x_disp_T.reshape(CC, 128, cap).transpose(1, 0, 2)   # [p, cc, t]
    return np.ascontiguousarray(a.reshape(128, CC * cap)
                                ).astype(ml_dtypes.bfloat16)


def _route(x, w_gate):
    """Host-side router. Runs the exact reference ops on jax-CPU so the
    top-2 selection and gate values match the reference bit-for-bit."""
    import jax
    import jax.numpy as jnp
    cpu = jax.devices("cpu")[0]
    with jax.default_device(cpu):
        xj = jnp.asarray(np.asarray(x))
        wj = jnp.asarray(np.asarray(w_gate))
        logits = jnp.einsum("btc,ce->bte", xj, wj)
        gates = jax.nn.softmax(logits, axis=-1)
        topk_vals, topk_idx = jax.lax.top_k(gates, TOP_K)
    return (np.asarray(topk_vals).reshape(-1, TOP_K),
            np.asarray(topk_idx).reshape(-1, TOP_K))


def _dispatch(x, topk_idx):
    """Token lists per expert, (token, slot) positions, cap, and the
    gathered+packed per-expert xtb slabs."""
    N = x.shape[0] * x.shape[1] if x.ndim == 3 else x.shape[0]
    x_flat = np.ascontiguousarray(np.asarray(x).reshape(N, C))
    idx_lists = []
    pos = np.empty((N, TOP_K), dtype=np.int64)
    for e in range(N_EXPERTS):
        sel = (topk_idx == e)
        toks = np.flatnonzero(sel.any(axis=1))
        idx_lists.append(toks)
        pos_of = np.full(N, -1, dtype=np.int64)
        pos_of[toks] = np.arange(len(toks))
        for k in range(TOP_K):
            m = sel[:, k]
            pos[m, k] = pos_of[m]
    max_cnt = max(len(t) for t in idx_lists)
    cap = max(128, -(-max_cnt // 4) * 4)

    xT = np.ascontiguousarray(x_flat.T)            # [C, N]
    xtbs = []
    for e in range(N_EXPERTS):
        toks = idx_lists[e]
        slab = np.zeros((C, cap), dtype=np.float32)
        slab[:, :len(toks)] = xT[:, toks]
        xtbs.append(pack_x(slab))
    return idx_lists, pos, cap, xtbs


def make_in_maps(x, wi, wg, wo, topk_idx):
    idx_lists, pos, cap, xtbs = _dispatch(x, topk_idx)
    in_maps = []
    for e in range(N_EXPERTS):
        in_maps.append({
            "xtb": xtbs[e],
            "wib": pack_wi(wi[e]),
            "wgb": pack_wi(wg[e]),
            "wob": pack_wo(wo[e]),
        })
    return idx_lists, pos, cap, in_maps


def kernel(x, w_gate, wi, wg, wo):
    x = np.asarray(x)
    wi, wg, wo = np.asarray(wi), np.asarray(wg), np.asarray(wo)
    N = B * T

    topk_vals, topk_idx = _route(x, w_gate)
    idx_lists, pos, cap, in_maps = make_in_maps(x, wi, wg, wo, topk_idx)

    nc = build_program(cap)
    res = run_bass_kernel_spmd(nc, in_maps, core_ids=list(range(N_CORES)))

    # combine: out[t] = sum_k vals[t,k] * y_{idx[t,k]}[t]
    Y = np.empty((N_EXPERTS, cap, C), dtype=np.float32)   # token-major
    for e in range(N_EXPERTS):
        Y[e] = res.results[e]["yt"].T
    out = (topk_vals[:, 0:1] * Y[topk_idx[:, 0], pos[:, 0], :]
           + topk_vals[:, 1:2] * Y[topk_idx[:, 1], pos[:, 1], :])
    return out.reshape(B, T, C).astype(np.float32)
